# revision 31
# baseline (speedup 1.0000x reference)
"""Trainium2 Bass kernel for nn_FFRFLayer (L2-norm -> LocalConv -> BatchNorm -> ReLU).

Self-contained: hardcodes shapes/sharding for
  x:(64,16,48,48) f32, kernel:(1936,400,128) f32, gamma/beta:(128,) f32
Output: (64,128,44,44) f32.

Sharding: 8 cores, each owns 6 output rows (cores 4-7: 5 valid + 1 zero-pad row).
Per core the LocalConv is computed as 5 accumulating matmuls per position
(kw-chunks of K=80=kh*16+c), with the patch operand read directly from an
(h*16+c)-partitioned x tile via plain slicing (no im2col materialization).
The unshared kernel is host-permuted to [p, kw, kh, c, cout] fp16 and streamed.
Sample L2 norms and BatchNorm stats use two tiny AllReduces across the cores.
"""

import numpy as np

import concourse.bass as bass
import concourse.tile as tile
from concourse import bacc, mybir
from concourse.bass_utils import run_bass_kernel_spmd

# ---------------- problem constants ----------------
B, C, H, W = 64, 16, 48, 48
KH = KW = 5
COUT = 128
OH = OW = 44
P_FULL = OH * OW          # 1936
F = C * KH * KW           # 400
EPS_NORM = 1e-7
EPS_BN = 1e-5

NCORES = 8
ROW_STARTS = [0, 6, 12, 18, 24, 29, 34, 39]
ROWS_VALID = [6, 6, 6, 6, 5, 5, 5, 5]
OHC = 6                   # output rows computed per core (padded)
PL = OHC * OW             # 264 local positions
NG = 3                    # row-pair groups per core
HG = 8                    # x rows per group tile (6 used + 2 pad)
XROWS = 12                # x rows staged per core
NROWS = 6                 # disjoint input rows per core for the L2 norm
K80 = KH * C              # 80  (contraction per kw-chunk, order kh*16+c)
PB = 8                    # positions per kernel DMA block
NBLK = PL // PB           # 33
NSTAT = float(B * OH * OW)

F16 = mybir.dt.float16
F32 = mybir.dt.float32


def build_program(use_collectives=True):
    # Bacc (not plain Bass): its compile() pass pipeline legalizes sync waits
    # (multi-wait DMAs are split), which walrus' DMA_DIRECT2D codegen requires.
    nc = bacc.Bacc(None, target_bir_lowering=False, num_devices=NCORES)

    xs = nc.dram_tensor("xs16", [XROWS * C, B, W], F16, kind="ExternalInput")
    xn = nc.dram_tensor("xnorm", [B, C * NROWS * W], F32, kind="ExternalInput")
    ks = nc.dram_tensor("kslab", [NBLK, K80, PB * KW * COUT], F16, kind="ExternalInput")
    gamma = nc.dram_tensor("gamma", [1, COUT], F32, kind="ExternalInput")
    beta = nc.dram_tensor("beta", [1, COUT], F32, kind="ExternalInput")
    out = nc.dram_tensor("out", [B, COUT, PL], F16, kind="ExternalOutput")

    with tile.TileContext(nc, num_cores=NCORES) as tc:
        with (
            tc.tile_pool(name="singles", bufs=1) as singles,
            tc.tile_pool(name="dram", bufs=1, space="DRAM") as dram,
            tc.tile_pool(name="xgpool", bufs=2) as xgpool,
            tc.tile_pool(name="normpool", bufs=2) as normpool,
            tc.tile_pool(name="kpool", bufs=2) as kpool,
            tc.tile_pool(name="psum", bufs=6, space="PSUM") as psumpool,
            tc.tile_pool(name="spsum", bufs=1, space="PSUM") as spsumpool,
            tc.tile_pool(name="stats", bufs=1) as statspool,
            tc.tile_pool(name="sqtmp", bufs=3) as sqpool,
            tc.tile_pool(name="osb", bufs=1) as osbpool,
            tc.tile_pool(name="apply", bufs=2) as applypool,
        ):
            # ---------- phase 0: L2 norm partial sums + AllReduce ----------
            NCH = 4
            CH = C * NROWS * W // NCH
            nrm_parts = singles.tile([B, NCH], F32)
            for ic in range(NCH):
                xn_sb = normpool.tile([B, CH], F32, tag="xn")
                nc.gpsimd.dma_start(out=xn_sb, in_=xn[:, ic * CH : (ic + 1) * CH])
                xsq = normpool.tile([B, CH], F16, tag="xsq")
                nc.scalar.activation(
                    out=xsq, in_=xn_sb,
                    func=mybir.ActivationFunctionType.Square,
                    accum_out=nrm_parts[:, ic : ic + 1],
                )
            nrm_part = singles.tile([B, 1], F32)
            nc.vector.tensor_reduce(
                out=nrm_part, in_=nrm_parts,
                op=mybir.AluOpType.add, axis=mybir.AxisListType.X,
            )

            # ---------- conv main loop ----------
            osb = osbpool.tile([B, COUT, PL], F16)
            accs = [statspool.tile([B, COUT, 4], F32, name=f"acc{i}") for i in range(2)]
            sqaccs = [statspool.tile([B, COUT, 4], F32, name=f"sqacc{i}") for i in range(2)]
            for t in (*accs, *sqaccs):
                nc.vector.memset(t, 0.0)

            xg_tiles = {}

            def get_xg(r):
                # rows r..r+KH flattened as (kh*C + c): exactly the 80
                # partitions this output row's matmuls contract over (base 0)
                if r not in xg_tiles:
                    xg = xgpool.tile([K80, B, W], F16, tag="xg")
                    nc.sync.dma_start(out=xg, in_=xs[r * C : r * C + K80, :, :])
                    xg_tiles[r] = xg
                return xg_tiles[r]

            pt = None
            for blk in range(NBLK):
                ktile = kpool.tile([K80, PB * KW * COUT], F16, tag="ktile")
                nc.sync.dma_start(out=ktile, in_=ks[blk, :, :])
                for pi in range(PB):
                    p = blk * PB + pi
                    oh_loc = p // OW
                    ow = p % OW
                    xg = get_xg(oh_loc)
                    if p % 4 == 0:
                        pt = psumpool.tile([B, 4, COUT], F32, tag="pt")
                    for kw in range(KW):
                        lhsT = xg[:, :, ow + kw]
                        rhs = ktile[:, (pi * KW + kw) * COUT : (pi * KW + kw + 1) * COUT]
                        nc.tensor.matmul(
                            pt[:, p % 4, :], lhsT, rhs,
                            start=(kw == 0), stop=(kw == KW - 1),
                        )
                    if p % 4 == 3:
                        g4 = p - 3
                        # evacuate raw conv output to the fp16 c-major store
                        # (the L2 norm scale s_b is folded into the BN apply
                        # coefficients); psum (p,c) read transposed to (c,p)
                        osl = osb[:, :, g4 : g4 + 4]
                        psrc = pt[:].rearrange("b f c -> b c f")
                        if (p // 4) % 2 == 0:
                            nc.vector.tensor_copy(osl, psrc)
                        else:  # alternate evacuation onto the ACT engine
                            nc.scalar.copy(osl, psrc)
                        # raw stats: sum on GPSIMD, squares on ACT + sum on
                        # DVE; two alternating accumulators break the serial
                        # read-modify-write chain
                        par = (p // 4) % 2
                        nc.gpsimd.tensor_add(accs[par], accs[par], osl)
                        sqt = sqpool.tile([B, COUT, 4], F16, tag="sqt")
                        nc.scalar.square(sqt, osl)
                        nc.vector.tensor_add(sqaccs[par], sqaccs[par], sqt)

            # ---------- stats fold + single fused AllReduce ----------
            stats2 = statspool.tile([B, 2 * COUT + 1], F32)
            nc.vector.tensor_add(accs[0], accs[0], accs[1])
            nc.vector.tensor_add(sqaccs[0], sqaccs[0], sqaccs[1])
            nc.vector.tensor_reduce(
                out=stats2[:, 0:COUT], in_=accs[0][:],
                op=mybir.AluOpType.add, axis=mybir.AxisListType.X,
            )
            nc.vector.tensor_reduce(
                out=stats2[:, COUT : 2 * COUT], in_=sqaccs[0][:],
                op=mybir.AluOpType.add, axis=mybir.AxisListType.X,
            )
            nc.vector.tensor_copy(stats2[:, 2 * COUT : 2 * COUT + 1], nrm_part)
            sb_in = dram.tile([B, 2 * COUT + 1], F32)
            sb_out = dram.tile([B, 2 * COUT + 1], F32)
            nc.gpsimd.dma_start(out=sb_in[:], in_=stats2)
            if use_collectives:
                nc.gpsimd.collective_compute(
                    "AllReduce",
                    mybir.AluOpType.add,
                    replica_groups=[list(range(NCORES))],
                    ins=[sb_in.opt()],
                    outs=[sb_out.opt()],
                )
            else:
                nc.gpsimd.dma_start(out=sb_out[:], in_=sb_in[:])
            gstat = statspool.tile([B, 2 * COUT + 1], F32)
            nc.gpsimd.dma_start(out=gstat, in_=sb_out[:])

            # global L2-norm scale per sample: s = 1/(sqrt(sumsq)+eps)
            s_sqrt = singles.tile([B, 1], F32)
            nc.scalar.sqrt(s_sqrt, gstat[:, 2 * COUT : 2 * COUT + 1])
            s_eps = singles.tile([B, 1], F32)
            nc.vector.tensor_scalar_add(s_eps, s_sqrt, EPS_NORM)
            s_col = singles.tile([B, 1], F32)
            nc.vector.reciprocal(s_col, s_eps)
            s2_col = singles.tile([B, 1], F32)
            nc.vector.tensor_mul(s2_col, s_col, s_col)

            # channel sums of the (normalized) output: S_c and SS_c
            st_ps = spsumpool.tile([1, 2 * COUT], F32)
            nc.tensor.matmul(st_ps[:, 0:COUT], s_col, gstat[:, 0:COUT],
                             start=True, stop=True)
            nc.tensor.matmul(st_ps[:, COUT : 2 * COUT], s2_col,
                             gstat[:, COUT : 2 * COUT], start=True, stop=True)
            glob = statspool.tile([1, 2 * COUT], F32)
            nc.vector.tensor_copy(glob, st_ps)

            # ---------- BN coefficients ----------
            gamma_sb = singles.tile([1, COUT], F32)
            beta_sb = singles.tile([1, COUT], F32)
            nc.gpsimd.dma_start(out=gamma_sb, in_=gamma[:, :])
            nc.gpsimd.dma_start(out=beta_sb, in_=beta[:, :])
            mu = statspool.tile([1, COUT], F32)
            nc.vector.tensor_scalar_mul(mu, glob[:, 0:COUT], 1.0 / NSTAT)
            e2 = statspool.tile([1, COUT], F32)
            nc.vector.tensor_scalar_mul(e2, glob[:, COUT : 2 * COUT], 1.0 / NSTAT)
            mu2 = statspool.tile([1, COUT], F32)
            nc.vector.tensor_mul(mu2, mu, mu)
            var = statspool.tile([1, COUT], F32)
            nc.vector.tensor_tensor(
                out=var, in0=e2, in1=mu2, op=mybir.AluOpType.subtract
            )
            varep = statspool.tile([1, COUT], F32)
            nc.vector.tensor_scalar_add(varep, var, EPS_BN)
            stdv = statspool.tile([1, COUT], F32)
            nc.scalar.sqrt(stdv, varep)
            rstd = statspool.tile([1, COUT], F32)
            nc.vector.reciprocal(rstd, stdv)
            a_vec = statspool.tile([1, COUT], F32)
            nc.vector.tensor_mul(a_vec, gamma_sb, rstd)
            mua = statspool.tile([1, COUT], F32)
            nc.vector.tensor_mul(mua, mu, a_vec)
            b_vec = statspool.tile([1, COUT], F32)
            nc.vector.tensor_tensor(
                out=b_vec, in0=beta_sb, in1=mua, op=mybir.AluOpType.subtract
            )

            # apply coefficients as per-partition scalars:
            # A2[b,c] = s_b * a_c (outer product via PE), B2[b,c] = b_c
            s_d = dram.tile([B, 1], F32)
            nc.gpsimd.dma_start(out=s_d[:], in_=s_col)
            s_row = singles.tile([1, B], F32)
            nc.gpsimd.dma_start(
                out=s_row, in_=s_d[:].rearrange("b one -> one b")
            )
            ones_row = singles.tile([1, B], F32)
            nc.vector.memset(ones_row, 1.0)
            sa_ps = spsumpool.tile([B, 2 * COUT], F32)
            nc.tensor.matmul(sa_ps[:, 0:COUT], s_row, a_vec, start=True, stop=True)
            nc.tensor.matmul(sa_ps[:, COUT : 2 * COUT], ones_row, b_vec,
                             start=True, stop=True)
            sa_sb = statspool.tile([B, 2 * COUT], F32)
            nc.vector.tensor_copy(sa_sb, sa_ps)

            # ---------- apply BN + ReLU, write out ----------
            # per-channel fused (x*A2[b,c] + B2[b,c]) via tensor_scalar (4x
            # DVE mode on packed fp16); ReLU rides the writeout DMA as a CCE
            # max against the zero-initialized output buffer.
            CB = COUT // 8  # 16 channels per writeout block
            for cb in range(8):
                y = applypool.tile([B, CB, PL], F16, tag="y")
                if cb % 8 < 5:  # DVE: per-c fused mult+add, one block-wide relu
                    for ci in range(CB):
                        c = cb * CB + ci
                        nc.vector.tensor_scalar(
                            out=y[:, ci, :],
                            in0=osb[:, c, :],
                            scalar1=sa_sb[:, c : c + 1],
                            scalar2=sa_sb[:, COUT + c : COUT + c + 1],
                            op0=mybir.AluOpType.mult,
                            op1=mybir.AluOpType.add,
                        )
                    nc.vector.tensor_scalar_max(y, y, 0.0)
                else:  # ACT: relu(x*a + b) fused per channel
                    for ci in range(CB):
                        c = cb * CB + ci
                        nc.scalar.activation(
                            y[:, ci, :], osb[:, c, :],
                            mybir.ActivationFunctionType.Relu,
                            bias=sa_sb[:, COUT + c : COUT + c + 1],
                            scale=sa_sb[:, c : c + 1],
                        )
                nc.sync.dma_start(out=out[:, cb * CB : (cb + 1) * CB, :], in_=y)

    nc.compile()
    return nc


def shard_inputs(x, kernel, gamma, beta):
    """Build the 8 per-core input maps (host-side preprocessing)."""
    x = np.asarray(x, np.float32)
    kernel = np.asarray(kernel, np.float32)
    # kernel -> [p, kw, kh, c, cout]
    k5 = kernel.reshape(P_FULL, C, KH, KW, COUT).transpose(0, 3, 2, 1, 4)
    in_maps = []
    for i in range(NCORES):
        r0 = ROW_STARTS[i]
        # x rows r0:r0+12, zero-padded, as (h, c, b, w) fp16
        xsl = np.zeros((XROWS, C, B, W), np.float16)
        hi = min(H, r0 + XROWS)
        xsl[: hi - r0] = x[:, :, r0:hi, :].transpose(2, 1, 0, 3)
        xs16 = np.ascontiguousarray(xsl.reshape(XROWS * C, B, W))
        # disjoint norm rows 6i:6i+6 as (b, c*6*w) fp32
        xnorm = np.ascontiguousarray(
            x[:, :, 6 * i : 6 * i + NROWS, :].reshape(B, C * NROWS * W)
        )
        # kernel slab: positions r0*44 ... +264 (zero-padded), to
        # [NBLK, (kh,c)=80, (pi,kw,cout)]
        p0 = r0 * OW
        npos = ROWS_VALID[i] * OW  # zero-pad positions beyond the rows this
        # core owns, so padded rows contribute nothing to the BN statistics
        ksl = np.zeros((PL, KW, KH, C, COUT), np.float16)
        ksl[:npos] = k5[p0 : p0 + npos]
        kslab = np.ascontiguousarray(
            ksl.reshape(NBLK, PB, KW, KH, C, COUT)
            .transpose(0, 3, 4, 1, 2, 5)
            .reshape(NBLK, K80, PB * KW * COUT)
        )
        in_maps.append(
            {
                "xs16": xs16,
                "xnorm": xnorm,
                "kslab": kslab,
                "gamma": np.ascontiguousarray(gamma, np.float32).reshape(1, COUT),
                "beta": np.ascontiguousarray(beta, np.float32).reshape(1, COUT),
            }
        )
    return in_maps


_cached = {}


def kernel(x, kernel, gamma, beta, _want_time=False):
    if "nc" not in _cached:
        _cached["nc"] = build_program()
    nc = _cached["nc"]
    in_maps = shard_inputs(x, kernel, gamma, beta)
    res = run_bass_kernel_spmd(nc, in_maps, core_ids=list(range(NCORES)))
    outs = [r["out"] for r in res.results]  # each (B, PL, COUT)
    full = np.zeros((B, COUT, OH, OW), np.float32)
    for i in range(NCORES):
        v = ROWS_VALID[i]
        r0 = ROW_STARTS[i]
        blockrows = outs[i][:, :, : v * OW].astype(np.float32)
        full[:, :, r0 : r0 + v, :] = blockrows.reshape(B, COUT, v, OW)
    if _want_time:
        return full, res
    return full


# revision 33
# speedup vs baseline: 1.0490x; 1.0490x over previous
"""Trainium2 Bass kernel for nn_FFRFLayer (L2-norm -> LocalConv -> BatchNorm -> ReLU).

Self-contained: hardcodes shapes/sharding for
  x:(64,16,48,48) f32, kernel:(1936,400,128) f32, gamma/beta:(128,) f32
Output: (64,128,44,44) f32.

Sharding: 8 cores, each owns 6 output rows (cores 4-7: 5 valid + 1 zero-pad row).
Per core the LocalConv is computed as 5 accumulating matmuls per position
(kw-chunks of K=80=kh*16+c), with the patch operand read directly from an
(h*16+c)-partitioned x tile via plain slicing (no im2col materialization).
The unshared kernel is host-permuted to [p, kw, kh, c, cout] fp16 and streamed.
Sample L2 norms and BatchNorm stats use two tiny AllReduces across the cores.
"""

import numpy as np

import concourse.bass as bass
import concourse.tile as tile
from concourse import bacc, mybir
from concourse.bass_utils import run_bass_kernel_spmd

# ---------------- problem constants ----------------
B, C, H, W = 64, 16, 48, 48
KH = KW = 5
COUT = 128
OH = OW = 44
P_FULL = OH * OW          # 1936
F = C * KH * KW           # 400
EPS_NORM = 1e-7
EPS_BN = 1e-5

NCORES = 8
ROW_STARTS = [0, 6, 12, 18, 24, 29, 34, 39]
ROWS_VALID = [6, 6, 6, 6, 5, 5, 5, 5]
OHC = 6                   # output rows computed per core (padded)
PL = OHC * OW             # 264 local positions
NG = 3                    # row-pair groups per core
HG = 8                    # x rows per group tile (6 used + 2 pad)
XROWS = 12                # x rows staged per core
NROWS = 6                 # disjoint input rows per core for the L2 norm
K80 = KH * C              # 80  (contraction per kw-chunk, order kh*16+c)
PB = 8                    # positions per kernel DMA block
NBLK = PL // PB           # 33
NSTAT = float(B * OH * OW)

F16 = mybir.dt.float16
F32 = mybir.dt.float32


def build_program(use_collectives=True):
    # Bacc (not plain Bass): its compile() pass pipeline legalizes sync waits
    # (multi-wait DMAs are split), which walrus' DMA_DIRECT2D codegen requires.
    nc = bacc.Bacc(None, target_bir_lowering=False, num_devices=NCORES)

    xs = nc.dram_tensor("xs16", [XROWS * C, B, W], F16, kind="ExternalInput")
    xn = nc.dram_tensor("xnorm", [B, C * NROWS * W], F32, kind="ExternalInput")
    ks = nc.dram_tensor("kslab", [NBLK, K80, PB * KW * COUT], F16, kind="ExternalInput")
    gamma = nc.dram_tensor("gamma", [1, COUT], F32, kind="ExternalInput")
    beta = nc.dram_tensor("beta", [1, COUT], F32, kind="ExternalInput")
    out = nc.dram_tensor("out", [B, COUT, PL], F16, kind="ExternalOutput")

    with tile.TileContext(nc, num_cores=NCORES) as tc:
        with (
            tc.tile_pool(name="singles", bufs=1) as singles,
            tc.tile_pool(name="dram", bufs=1, space="DRAM") as dram,
            tc.tile_pool(name="xgpool", bufs=2) as xgpool,
            tc.tile_pool(name="normpool", bufs=2) as normpool,
            tc.tile_pool(name="kpool", bufs=2) as kpool,
            tc.tile_pool(name="psum", bufs=6, space="PSUM") as psumpool,
            tc.tile_pool(name="spsum", bufs=1, space="PSUM") as spsumpool,
            tc.tile_pool(name="stats", bufs=1) as statspool,
            tc.tile_pool(name="sqtmp", bufs=3) as sqpool,
            tc.tile_pool(name="osb", bufs=1) as osbpool,
            tc.tile_pool(name="apply", bufs=2) as applypool,
        ):
            # ---------- phase 0: L2 norm partial sums + AllReduce ----------
            NCH = 4
            CH = C * NROWS * W // NCH
            nrm_parts = singles.tile([B, NCH], F32)
            for ic in range(NCH):
                xn_sb = normpool.tile([B, CH], F32, tag="xn")
                nc.gpsimd.dma_start(out=xn_sb, in_=xn[:, ic * CH : (ic + 1) * CH])
                xsq = normpool.tile([B, CH], F16, tag="xsq")
                nc.scalar.activation(
                    out=xsq, in_=xn_sb,
                    func=mybir.ActivationFunctionType.Square,
                    accum_out=nrm_parts[:, ic : ic + 1],
                )
            nrm_part = singles.tile([B, 1], F32)
            nc.vector.tensor_reduce(
                out=nrm_part, in_=nrm_parts,
                op=mybir.AluOpType.add, axis=mybir.AxisListType.X,
            )

            # ---------- conv main loop ----------
            osb = osbpool.tile([B, COUT, PL], F16)
            accs = [statspool.tile([B, COUT, 4], F32, name=f"acc{i}") for i in range(2)]
            sqaccs = [statspool.tile([B, COUT, 4], F32, name=f"sqacc{i}") for i in range(2)]
            for t in (*accs, *sqaccs):
                nc.vector.memset(t, 0.0)

            xg_tiles = {}

            def get_xg(r):
                # rows r..r+KH flattened as (kh*C + c): exactly the 80
                # partitions this output row's matmuls contract over (base 0)
                if r not in xg_tiles:
                    xg = xgpool.tile([K80, B, W], F16, tag="xg")
                    nc.sync.dma_start(out=xg, in_=xs[r * C : r * C + K80, :, :])
                    xg_tiles[r] = xg
                return xg_tiles[r]

            pt = None
            for blk in range(NBLK):
                ktile = kpool.tile([K80, PB * KW * COUT], F16, tag="ktile")
                nc.sync.dma_start(out=ktile, in_=ks[blk, :, :])
                for pi in range(PB):
                    p = blk * PB + pi
                    oh_loc = p // OW
                    ow = p % OW
                    xg = get_xg(oh_loc)
                    if p % 4 == 0:
                        pt = psumpool.tile([B, 4, COUT], F32, tag="pt")
                    for kw in range(KW):
                        lhsT = xg[:, :, ow + kw]
                        rhs = ktile[:, (pi * KW + kw) * COUT : (pi * KW + kw + 1) * COUT]
                        nc.tensor.matmul(
                            pt[:, p % 4, :], lhsT, rhs,
                            start=(kw == 0), stop=(kw == KW - 1),
                        )
                    if p % 4 == 3:
                        g4 = p - 3
                        # evacuate raw conv output to the fp16 c-major store
                        # (the L2 norm scale s_b is folded into the BN apply
                        # coefficients); psum (p,c) read transposed to (c,p)
                        osl = osb[:, :, g4 : g4 + 4]
                        psrc = pt[:].rearrange("b f c -> b c f")
                        if (p // 4) % 2 == 0:
                            nc.vector.tensor_copy(osl, psrc)
                        else:  # alternate evacuation onto the ACT engine
                            nc.scalar.copy(osl, psrc)
                        # raw stats: sum on GPSIMD, squares on ACT + sum on
                        # DVE; two alternating accumulators break the serial
                        # read-modify-write chain
                        par = (p // 4) % 2
                        nc.gpsimd.tensor_add(accs[par], accs[par], osl)
                        sqt = sqpool.tile([B, COUT, 4], F16, tag="sqt")
                        nc.scalar.square(sqt, osl)
                        nc.vector.tensor_add(sqaccs[par], sqaccs[par], sqt)

            # ---------- stats fold + single fused AllReduce ----------
            stats2 = statspool.tile([B, 2 * COUT + 1], F32)
            nc.vector.tensor_add(accs[0], accs[0], accs[1])
            nc.vector.tensor_add(sqaccs[0], sqaccs[0], sqaccs[1])
            nc.vector.tensor_reduce(
                out=stats2[:, 0:COUT], in_=accs[0][:],
                op=mybir.AluOpType.add, axis=mybir.AxisListType.X,
            )
            nc.vector.tensor_reduce(
                out=stats2[:, COUT : 2 * COUT], in_=sqaccs[0][:],
                op=mybir.AluOpType.add, axis=mybir.AxisListType.X,
            )
            nc.vector.tensor_copy(stats2[:, 2 * COUT : 2 * COUT + 1], nrm_part)
            sb_in = dram.tile([B, 2 * COUT + 1], F32)
            sb_out = dram.tile([B, 2 * COUT + 1], F32)
            nc.gpsimd.dma_start(out=sb_in[:], in_=stats2)
            if use_collectives:
                nc.gpsimd.collective_compute(
                    "AllReduce",
                    mybir.AluOpType.add,
                    replica_groups=[list(range(NCORES))],
                    ins=[sb_in.opt()],
                    outs=[sb_out.opt()],
                )
            else:
                nc.gpsimd.dma_start(out=sb_out[:], in_=sb_in[:])
            gstat = statspool.tile([B, 2 * COUT + 1], F32)
            nc.gpsimd.dma_start(out=gstat, in_=sb_out[:])

            # global L2-norm scale per sample: s = 1/(sqrt(sumsq)+eps)
            s_sqrt = singles.tile([B, 1], F32)
            nc.scalar.sqrt(s_sqrt, gstat[:, 2 * COUT : 2 * COUT + 1])
            s_eps = singles.tile([B, 1], F32)
            nc.vector.tensor_scalar_add(s_eps, s_sqrt, EPS_NORM)
            s_col = singles.tile([B, 1], F32)
            nc.vector.reciprocal(s_col, s_eps)
            s2_col = singles.tile([B, 1], F32)
            nc.vector.tensor_mul(s2_col, s_col, s_col)

            # channel sums of the (normalized) output: S_c and SS_c
            st_ps = spsumpool.tile([1, 2 * COUT], F32)
            nc.tensor.matmul(st_ps[:, 0:COUT], s_col, gstat[:, 0:COUT],
                             start=True, stop=True)
            nc.tensor.matmul(st_ps[:, COUT : 2 * COUT], s2_col,
                             gstat[:, COUT : 2 * COUT], start=True, stop=True)
            glob = statspool.tile([1, 2 * COUT], F32)
            nc.vector.tensor_copy(glob, st_ps)

            # ---------- BN coefficients ----------
            gamma_sb = singles.tile([1, COUT], F32)
            beta_sb = singles.tile([1, COUT], F32)
            nc.gpsimd.dma_start(out=gamma_sb, in_=gamma[:, :])
            nc.gpsimd.dma_start(out=beta_sb, in_=beta[:, :])
            mu = statspool.tile([1, COUT], F32)
            nc.vector.tensor_scalar_mul(mu, glob[:, 0:COUT], 1.0 / NSTAT)
            e2 = statspool.tile([1, COUT], F32)
            nc.vector.tensor_scalar_mul(e2, glob[:, COUT : 2 * COUT], 1.0 / NSTAT)
            mu2 = statspool.tile([1, COUT], F32)
            nc.vector.tensor_mul(mu2, mu, mu)
            var = statspool.tile([1, COUT], F32)
            nc.vector.tensor_tensor(
                out=var, in0=e2, in1=mu2, op=mybir.AluOpType.subtract
            )
            varep = statspool.tile([1, COUT], F32)
            nc.vector.tensor_scalar_add(varep, var, EPS_BN)
            stdv = statspool.tile([1, COUT], F32)
            nc.scalar.sqrt(stdv, varep)
            rstd = statspool.tile([1, COUT], F32)
            nc.vector.reciprocal(rstd, stdv)
            a_vec = statspool.tile([1, COUT], F32)
            nc.vector.tensor_mul(a_vec, gamma_sb, rstd)
            mua = statspool.tile([1, COUT], F32)
            nc.vector.tensor_mul(mua, mu, a_vec)
            b_vec = statspool.tile([1, COUT], F32)
            nc.vector.tensor_tensor(
                out=b_vec, in0=beta_sb, in1=mua, op=mybir.AluOpType.subtract
            )

            # apply coefficients as per-partition scalars:
            # A2[b,c] = s_b * a_c (outer product via PE), B2[b,c] = b_c
            s_d = dram.tile([B, 1], F32)
            nc.gpsimd.dma_start(out=s_d[:], in_=s_col)
            s_row = singles.tile([1, B], F32)
            nc.gpsimd.dma_start(
                out=s_row, in_=s_d[:].rearrange("b one -> one b")
            )
            ones_row = singles.tile([1, B], F32)
            nc.vector.memset(ones_row, 1.0)
            sa_ps = spsumpool.tile([B, 2 * COUT], F32)
            nc.tensor.matmul(sa_ps[:, 0:COUT], s_row, a_vec, start=True, stop=True)
            nc.tensor.matmul(sa_ps[:, COUT : 2 * COUT], ones_row, b_vec,
                             start=True, stop=True)
            sa_sb = statspool.tile([B, 2 * COUT], F32)
            nc.vector.tensor_copy(sa_sb, sa_ps)

            # ---------- apply BN + ReLU, write out ----------
            # per-channel fused (x*A2[b,c] + B2[b,c]) via tensor_scalar (4x
            # DVE mode on packed fp16); ReLU rides the writeout DMA as a CCE
            # max against the zero-initialized output buffer.
            CB = COUT // 8  # 16 channels per writeout block
            for cb in range(8):
                y = applypool.tile([B, CB, PL], F16, tag="y")
                if cb % 8 < 8:  # DVE: per-c fused mult+add, one block-wide relu
                    for ci in range(CB):
                        c = cb * CB + ci
                        nc.vector.tensor_scalar(
                            out=y[:, ci, :],
                            in0=osb[:, c, :],
                            scalar1=sa_sb[:, c : c + 1],
                            scalar2=sa_sb[:, COUT + c : COUT + c + 1],
                            op0=mybir.AluOpType.mult,
                            op1=mybir.AluOpType.add,
                        )
                    nc.vector.tensor_scalar_max(y, y, 0.0)
                else:  # ACT: relu(x*a + b) fused per channel
                    for ci in range(CB):
                        c = cb * CB + ci
                        nc.scalar.activation(
                            y[:, ci, :], osb[:, c, :],
                            mybir.ActivationFunctionType.Relu,
                            bias=sa_sb[:, COUT + c : COUT + c + 1],
                            scale=sa_sb[:, c : c + 1],
                        )
                nc.sync.dma_start(out=out[:, cb * CB : (cb + 1) * CB, :], in_=y)

    nc.compile()
    return nc


def shard_inputs(x, kernel, gamma, beta):
    """Build the 8 per-core input maps (host-side preprocessing)."""
    x = np.asarray(x, np.float32)
    kernel = np.asarray(kernel, np.float32)
    # kernel -> [p, kw, kh, c, cout]
    k5 = kernel.reshape(P_FULL, C, KH, KW, COUT).transpose(0, 3, 2, 1, 4)
    in_maps = []
    for i in range(NCORES):
        r0 = ROW_STARTS[i]
        # x rows r0:r0+12, zero-padded, as (h, c, b, w) fp16
        xsl = np.zeros((XROWS, C, B, W), np.float16)
        hi = min(H, r0 + XROWS)
        xsl[: hi - r0] = x[:, :, r0:hi, :].transpose(2, 1, 0, 3)
        xs16 = np.ascontiguousarray(xsl.reshape(XROWS * C, B, W))
        # disjoint norm rows 6i:6i+6 as (b, c*6*w) fp32
        xnorm = np.ascontiguousarray(
            x[:, :, 6 * i : 6 * i + NROWS, :].reshape(B, C * NROWS * W)
        )
        # kernel slab: positions r0*44 ... +264 (zero-padded), to
        # [NBLK, (kh,c)=80, (pi,kw,cout)]
        p0 = r0 * OW
        npos = ROWS_VALID[i] * OW  # zero-pad positions beyond the rows this
        # core owns, so padded rows contribute nothing to the BN statistics
        ksl = np.zeros((PL, KW, KH, C, COUT), np.float16)
        ksl[:npos] = k5[p0 : p0 + npos]
        kslab = np.ascontiguousarray(
            ksl.reshape(NBLK, PB, KW, KH, C, COUT)
            .transpose(0, 3, 4, 1, 2, 5)
            .reshape(NBLK, K80, PB * KW * COUT)
        )
        in_maps.append(
            {
                "xs16": xs16,
                "xnorm": xnorm,
                "kslab": kslab,
                "gamma": np.ascontiguousarray(gamma, np.float32).reshape(1, COUT),
                "beta": np.ascontiguousarray(beta, np.float32).reshape(1, COUT),
            }
        )
    return in_maps


_cached = {}


def kernel(x, kernel, gamma, beta, _want_time=False):
    if "nc" not in _cached:
        _cached["nc"] = build_program()
    nc = _cached["nc"]
    in_maps = shard_inputs(x, kernel, gamma, beta)
    res = run_bass_kernel_spmd(nc, in_maps, core_ids=list(range(NCORES)))
    outs = [r["out"] for r in res.results]  # each (B, PL, COUT)
    full = np.zeros((B, COUT, OH, OW), np.float32)
    for i in range(NCORES):
        v = ROWS_VALID[i]
        r0 = ROW_STARTS[i]
        blockrows = outs[i][:, :, : v * OW].astype(np.float32)
        full[:, :, r0 : r0 + v, :] = blockrows.reshape(B, COUT, v, OW)
    if _want_time:
        return full, res
    return full
